# revision 3
# baseline (speedup 1.0000x reference)
"""Trainium2 Bass kernel for MoE head adapter (top-2 of 4 experts + proj).

Sparse expert-pair dispatch version.

Per core (8192 tokens, weights replicated, data-parallel over 8 cores):
  Phase A (gating+routing, batched over all 64 token-blocks):
    - exact-f32 gating logits via PE matmuls (tokens on partitions)
    - top-2 mask, softmax-of-2 -> hi-expert gate (lo = 1 - hi)
    - pair key in {1..6} via dot(mask, [0,1,2,4])
    - global rank within pair region: inclusive scan along free axis
      (tensor_tensor_scan) + strict-upper-triangular matmul for the
      partition prefix; slot = region_base - 1 + prefix + incl_scan
    - one indirect DMA scatters (token_id, hi_gate) f32 pairs into
      perm_meta[slot] (pads stay -1 from an init fill)
  Phase B (19 supertiles of 512 slots, pair regions 256-aligned):
    - load meta block; idx16 = max(token,0) wrap-16 replicated
    - dma_gather(transpose=True): gathers 512 token rows of x (bf16)
      AND transposes into [128, kd, 512] xT layout in one DMA
    - G_hi broadcast via ones-matmul; G_lo = 1 - G_hi
    - 2-expert up-proj / gelu / gate-mult / down-proj (bf16 matmuls),
      column-split at the <=1 pair boundary inside a supertile
    - output projection; dense bf16 write to perm_out[slot]
  Host: unpermute perm_out rows via perm_meta token ids, upcast, + b_proj.
"""

import os
from contextlib import ExitStack

import numpy as np

import concourse.bass as bass
import concourse.tile as tile
from concourse import bacc, mybir
from concourse.bass import IndirectOffsetOnAxis
from concourse.bass_utils import run_bass_kernel_spmd

N, D, E, H, EMB = 65536, 256, 4, 512, 512
NCORES = 8
NSH = N // NCORES           # tokens per core
KD = D // 128               # k-tiles over D
MH = H // 128               # m-tiles over H
SUPER = 512
NSUP = NSH // SUPER         # gating supertiles (16)
NSB = NSH // 128            # 128-token blocks (64)

# expert-pair regions, ordered by pair key (dot(mask, [0,1,2,4]) = j+1)
PAIRS = [(0, 1), (0, 2), (1, 2), (0, 3), (1, 3), (2, 3)]
CAPS = [2304, 768, 1792, 1792, 768, 2304]
STARTS = [0]
for c in CAPS:
    STARTS.append(STARTS[-1] + c)
CAP_TOT = STARTS[-1]        # 9728
NST = CAP_TOT // SUPER      # 19 supertiles


def _runs_for_st(s):
    """[(pair_j, col_off, col_len)] covering slots [512s, 512s+512)."""
    lo, hi = s * SUPER, (s + 1) * SUPER
    runs = []
    for j in range(6):
        a, b = max(lo, STARTS[j]), min(hi, STARTS[j + 1])
        if a < b:
            runs.append((j, a - lo, b - a))
    return runs


RUNS = [_runs_for_st(s) for s in range(NST)]

F32 = mybir.dt.float32
I32 = mybir.dt.int32
I16 = mybir.dt.int16
BF16 = mybir.dt.bfloat16
AF = mybir.ActivationFunctionType
ALU = mybir.AluOpType
AX = mybir.AxisListType
MM_DT = BF16


def _moe_body(ctx: ExitStack, tc, xt, xbf, wg, w1, w2, wp, u128, ident, r16, pmeta, pout):
    nc = tc.nc

    const = ctx.enter_context(tc.tile_pool(name="const", bufs=1))
    sb = ctx.enter_context(tc.tile_pool(name="sb", bufs=2))
    ps = ctx.enter_context(tc.tile_pool(name="ps", bufs=1, space="PSUM"))

    # --- replicated weights -------------------------------------------------
    w1_sb = const.tile([128, KD, E, H], MM_DT)
    w2_sb = const.tile([128, MH, E, D], MM_DT)
    wp_sb = const.tile([128, KD, EMB], MM_DT)
    wg_sb = const.tile([128, KD, E], F32)
    for k in range(KD):
        for e in range(E):
            nc.gpsimd.dma_start(w1_sb[:, k, e, :], w1[e, k * 128:(k + 1) * 128, :])
    for m in range(MH):
        for e in range(E):
            nc.gpsimd.dma_start(w2_sb[:, m, e, :], w2[e, m * 128:(m + 1) * 128, :])
    nc.gpsimd.dma_start(wp_sb[:], wp.rearrange("(k p) m -> p k m", p=128))
    nc.gpsimd.dma_start(wg_sb[:], wg.rearrange("(k p) e -> p k e", p=128))
    u128_sb = const.tile([128, 128], F32)
    nc.gpsimd.dma_start(u128_sb[:], u128[:])
    ident_sb = const.tile([128, 128], F32)
    nc.gpsimd.dma_start(ident_sb[:], ident[:])
    r16_sb = const.tile([16, 128], F32)
    nc.gpsimd.dma_start(r16_sb[:], r16[:])

    # --- small constants ----------------------------------------------------
    ones_bf = const.tile([1, 128], MM_DT)
    nc.vector.memset(ones_bf[:], 1.0)
    ones_f = const.tile([1, 128], F32)
    nc.vector.memset(ones_f[:], 1.0)
    basev = const.tile([1, 6], F32)
    for j in range(6):
        nc.vector.memset(basev[:, j : j + 1], float(STARTS[j] - 1))
    cvals = const.tile([128, 1, 4], F32)
    for e, cv in enumerate((0.0, 1.0, 2.0, 4.0)):
        nc.vector.memset(cvals[:, :, e : e + 1], cv)
    # token ids stored +1 so the zero-initialized pad slots read as -1 later
    tok_i = const.tile([128, NSB], I32)
    nc.gpsimd.iota(tok_i[:], pattern=[[128, NSB]], base=1, channel_multiplier=1)
    tok_f = const.tile([128, NSB], F32)
    nc.vector.tensor_copy(tok_f[:], tok_i[:])
    ev_i = const.tile([128, 4], I32)
    nc.gpsimd.iota(ev_i[:], pattern=[[1, 4]], base=0, channel_multiplier=0)
    evals = const.tile([128, 1, 4], F32)
    nc.vector.tensor_copy(evals[:, 0, :], ev_i[:])
    jv_i = const.tile([128, 6], I32)
    nc.gpsimd.iota(jv_i[:], pattern=[[1, 6]], base=1, channel_multiplier=0)
    jvals = const.tile([128, 6, 1], F32)
    nc.vector.tensor_copy(jvals[:, :, 0], jv_i[:])

    # --- zero-init perm_meta (scatter-add target; pads stay 0) --------------
    zin = const.tile([128, 608], F32)
    nc.vector.memset(zin[:], 0.0)
    pm_flat = pmeta.rearrange("(p a) c -> p (a c)", p=128)
    for b in range(8):
        nc.sync.dma_start(pm_flat[:, b * 608 : (b + 1) * 608], zin[:])

    # ===================== Phase A: gating + routing =========================
    lg_ps = ps.tile([128, NSB, E], F32, tag="h", bufs=2)
    for T in range(NSUP):
        xt32 = sb.tile([128, KD, SUPER], F32, tag="xt32", bufs=3)
        nc.sync.dma_start(
            xt32[:],
            xt[:, T * SUPER : (T + 1) * SUPER].rearrange("(k p) t -> p k t", p=128),
        )
        for s4 in range(SUPER // 128):
            s = T * 4 + s4
            for k in range(KD):
                nc.tensor.matmul(
                    lg_ps[:, s, :],
                    xt32[:, k, s4 * 128 : (s4 + 1) * 128],
                    wg_sb[:, k, :],
                    start=(k == 0),
                    stop=(k == KD - 1),
                )

    def bc(t, shape):
        return t[:].broadcast_to(shape)

    SH = [128, NSB, E]
    lg = sb.tile(SH, F32, tag="lg_sb")
    nc.vector.tensor_copy(lg[:], lg_ps[:])
    m1 = sb.tile([128, NSB, 1], F32, tag="m1")
    nc.vector.reduce_max(m1[:], lg[:], axis=AX.X)
    t0 = sb.tile(SH, F32, tag="t0")
    nc.vector.tensor_tensor(t0[:], lg[:], bc(m1, SH), op=ALU.is_equal)
    t1 = sb.tile(SH, F32, tag="t1")
    nc.vector.tensor_scalar_mul(t1[:], t0[:], -1e9)
    t2 = sb.tile(SH, F32, tag="t2")
    nc.vector.tensor_tensor(t2[:], lg[:], t1[:], op=ALU.add)
    m2 = sb.tile([128, NSB, 1], F32, tag="m2")
    nc.vector.reduce_max(m2[:], t2[:], axis=AX.X)
    mk = sb.tile(SH, F32, tag="mk")
    nc.vector.tensor_tensor(mk[:], lg[:], bc(m2, SH), op=ALU.is_ge)
    t4 = sb.tile(SH, F32, tag="t4")
    nc.vector.tensor_tensor(t4[:], lg[:], bc(m1, SH), op=ALU.subtract)
    t5 = sb.tile(SH, F32, tag="t5")
    nc.scalar.activation(t5[:], t4[:], AF.Exp)
    t6 = sb.tile(SH, F32, tag="t6")
    nc.vector.tensor_tensor(t6[:], t5[:], mk[:], op=ALU.mult)
    den = sb.tile([128, NSB, 1], F32, tag="den")
    nc.vector.reduce_sum(den[:], t6[:], axis=AX.X)
    rcp = sb.tile([128, NSB, 1], F32, tag="rcp")
    nc.vector.reciprocal(rcp[:], den[:])
    # hi-expert (higher index of the selected pair) gate
    em = sb.tile(SH, F32, tag="em")
    nc.vector.tensor_tensor(em[:], mk[:], bc(evals, SH), op=ALU.mult)
    bh = sb.tile([128, NSB, 1], F32, tag="bh")
    nc.vector.reduce_max(bh[:], em[:], axis=AX.X)
    hm = sb.tile(SH, F32, tag="hm")
    nc.vector.tensor_tensor(hm[:], em[:], bc(bh, SH), op=ALU.is_equal)
    t7 = sb.tile(SH, F32, tag="t7")
    nc.vector.tensor_tensor(t7[:], t6[:], hm[:], op=ALU.mult)
    hs = sb.tile([128, NSB, 1], F32, tag="hs")
    nc.vector.reduce_sum(hs[:], t7[:], axis=AX.X)
    hig = sb.tile([128, NSB, 1], F32, tag="hig")
    nc.vector.tensor_tensor(hig[:], hs[:], rcp[:], op=ALU.mult)
    # pair key (1..6)
    t8 = sb.tile(SH, F32, tag="t8")
    nc.vector.tensor_tensor(t8[:], mk[:], bc(cvals, SH), op=ALU.mult)
    pk = sb.tile([128, NSB, 1], F32, tag="pk")
    nc.vector.reduce_sum(pk[:], t8[:], axis=AX.X)
    # pair indicator [128, 6, NSB]
    IS = [128, 6, NSB]
    I_t = sb.tile(IS, F32, tag="I")
    nc.vector.tensor_tensor(
        I_t[:],
        pk[:, None, :, 0].broadcast_to(IS),
        bc(jvals, IS),
        op=ALU.is_equal,
    )
    # inclusive scan along the free (block) axis, per pair lane
    incl = sb.tile(IS, F32, tag="incl")
    for j in range(6):
        nc.vector.tensor_tensor_scan(
            incl[:, j, :], I_t[:, j, :], I_t[:, j, :], 0.0, op0=ALU.add, op1=ALU.bypass
        )
    # partition-exclusive prefix of lane rowsums + (base - 1)
    pfx_ps = ps.tile([128, 6], F32, tag="G")
    nc.tensor.matmul(pfx_ps[:], u128_sb[:], incl[:, :, NSB - 1], start=True, stop=False)
    nc.tensor.matmul(pfx_ps[:], ones_f[:], basev[:], start=False, stop=True)
    pfx = sb.tile([128, 6], F32, tag="pfxsb")
    nc.vector.tensor_copy(pfx[:], pfx_ps[:])
    sstuff = sb.tile(IS, F32, tag="sstuff")
    nc.vector.tensor_tensor(
        sstuff[:], incl[:], pfx[:, :, None].broadcast_to(IS), op=ALU.add
    )
    sel = sb.tile(IS, F32, tag="sel")
    nc.vector.tensor_tensor(sel[:], sstuff[:], I_t[:], op=ALU.mult)
    s3 = sb.tile([128, 3, NSB], F32, tag="s3")
    nc.vector.tensor_tensor(s3[:], sel[:, 0:3, :], sel[:, 3:6, :], op=ALU.add)
    s2 = sb.tile([128, NSB], F32, tag="s2")
    nc.vector.tensor_tensor(s2[:], s3[:, 0, :], s3[:, 1, :], op=ALU.add)
    slot_f = sb.tile([128, NSB], F32, tag="slotf")
    nc.vector.tensor_tensor(slot_f[:], s2[:], s3[:, 2, :], op=ALU.add)
    # meta rows (token_id+1, hi_gate)
    meta = sb.tile([128, NSB, 2], F32, tag="meta")
    nc.vector.tensor_copy(meta[:, :, 0], tok_f[:])
    nc.vector.tensor_copy(meta[:, :, 1], hig[:, :, 0])
    # Build the scatter index list in 16-partition wrap order: position
    # i = s*128 + p must sit at [i%16, i//16] = [p%16, s*8 + p//16].
    # Two transpose stages move slot[p, s] -> idxT[q, j, s] (p = j*16+q),
    # then a strided copy reorders (j, s) -> (s, j), and an R16 matmul
    # replicates the 16 live partitions across all 128.
    slotT_ps = ps.tile([64, 128], F32, tag="G")
    nc.tensor.transpose(slotT_ps[:], slot_f[:], ident_sb[:])
    slotT = sb.tile([64, 128], F32, tag="slotT")
    nc.vector.tensor_copy(slotT[:], slotT_ps[:])
    idxT_ps = ps.tile([16, 8, NSB], F32, tag="o")
    for j in range(8):
        nc.tensor.transpose(
            idxT_ps[:, j, :], slotT[:, j * 16 : (j + 1) * 16], ident_sb[:64, :64]
        )
    idx_sw = sb.tile([16, NSB, 8], F32, tag="idxsw")
    nc.vector.tensor_copy(idx_sw[:], idxT_ps[:].rearrange("q j s -> q s j"))
    rep_ps = ps.tile([128, NSB * 8], F32, tag="h", bufs=2)
    nc.tensor.matmul(
        rep_ps[:], r16_sb[:], idx_sw[:].rearrange("q s j -> q (s j)"),
        start=True, stop=True,
    )
    idx16s = sb.tile([128, NSB * 8], I16, tag="idx16s")
    nc.vector.tensor_copy(idx16s[:], rep_ps[:])
    # SWDGE ring fits <=128 descriptors per direction per op (m2s = n/8+1),
    # so scatter in chunks of 896 indices (7 s-columns each).
    CH = 7
    for c0 in range(0, NSB, CH):
        cw = min(CH, NSB - c0)
        n = cw * 128
        nc.gpsimd.dma_scatter_add(
            pmeta[:, 0:2],
            meta[:, c0 : c0 + cw, :],
            idx16s[:, c0 * 8 : (c0 + cw) * 8],
            n,
            n,
            2,
            elem_step=64,
        )

    # ===================== Phase B: experts + proj ===========================
    for s in range(NST):
        sl0 = s * SUPER
        # --- permuted meta loads (token+1 at [i%16, i//16] for slot sl0+i)
        idxf = sb.tile([16, SUPER // 16], F32, tag="idxf")
        nc.sync.dma_start(
            idxf[:],
            pmeta[sl0 : sl0 + SUPER, 0:1].rearrange("(c p) one -> p (c one)", p=16),
        )
        irep_ps = ps.tile([128, SUPER // 16], F32, tag="o")
        nc.tensor.matmul(irep_ps[:], r16_sb[:], idxf[:], start=True, stop=True)
        idxm = sb.tile([128, SUPER // 16], F32, tag="idxm")
        nc.vector.tensor_scalar(
            idxm[:], irep_ps[:], 1.0, 0.0, op0=ALU.subtract, op1=ALU.max
        )
        idx16 = sb.tile([128, SUPER // 16], I16, tag="idx16")
        nc.vector.tensor_copy(idx16[:], idxm[:])
        hi_row = sb.tile([1, SUPER], F32, tag="hirow")
        nc.sync.dma_start(
            hi_row[:], pmeta[sl0 : sl0 + SUPER, 1:2].rearrange("t one -> one t")
        )
        hi_bf = sb.tile([1, SUPER], MM_DT, tag="hibf")
        nc.vector.tensor_copy(hi_bf[:], hi_row[:])
        # --- gather + transpose x rows for this supertile
        xt_sb = sb.tile([128, KD, SUPER], MM_DT, tag="xtg", bufs=3)
        nc.gpsimd.dma_gather(
            xt_sb[:], xbf[:, :], idx16[:], SUPER, SUPER, D, transpose=True
        )
        # --- token gates broadcast over partitions
        g_ps = ps.tile([128, SUPER], F32, tag="G")
        nc.tensor.matmul(g_ps[:], ones_bf[:], hi_bf[:], start=True, stop=True)
        g_hi = sb.tile([128, SUPER], MM_DT, tag="ghi")
        nc.vector.tensor_copy(g_hi[:], g_ps[:])
        g_lo = sb.tile([128, SUPER], MM_DT, tag="glo")
        nc.vector.tensor_scalar(g_lo[:], g_ps[:], -1.0, 1.0, op0=ALU.mult, op1=ALU.add)
        g_es = (g_lo, g_hi)

        # --- experts (2 per token, split at pair boundaries)
        hgg = [
            sb.tile([128, MH, SUPER], MM_DT, tag=f"hgg{es}", name=f"hgg{s}_{es}")
            for es in range(2)
        ]
        for j, off, ln in RUNS[s]:
            pair = PAIRS[j]
            for es in range(2):
                e = pair[es]
                for mp in range(MH // 2):
                    h_ps = ps.tile(
                        [128, 2, SUPER], F32, tag="h", bufs=2, name=f"h{s}_{j}_{es}_{mp}"
                    )
                    for mm in range(2):
                        m = 2 * mp + mm
                        for k in range(KD):
                            nc.tensor.matmul(
                                h_ps[:, mm, off : off + ln],
                                w1_sb[:, k, e, m * 128 : (m + 1) * 128],
                                xt_sb[:, k, off : off + ln],
                                start=(k == 0),
                                stop=(k == KD - 1),
                            )
                    hg = sb.tile([128, 2, SUPER], MM_DT, tag="hg", bufs=3)
                    nc.scalar.activation(
                        hg[:, :, off : off + ln], h_ps[:, :, off : off + ln], AF.Gelu
                    )
                    nc.vector.tensor_tensor(
                        hgg[es][:, 2 * mp : 2 * mp + 2, off : off + ln],
                        hg[:, :, off : off + ln],
                        g_es[es][:, None, off : off + ln].broadcast_to([128, 2, ln]),
                        op=ALU.mult,
                    )
        yt_ps = ps.tile([128, KD, SUPER], F32, tag="yt")
        nruns = len(RUNS[s])
        for ri, (j, off, ln) in enumerate(RUNS[s]):
            pair = PAIRS[j]
            for es in range(2):
                e = pair[es]
                for md in range(KD):
                    for m in range(MH):
                        nc.tensor.matmul(
                            yt_ps[:, md, off : off + ln],
                            w2_sb[:, m, e, md * 128 : (md + 1) * 128],
                            hgg[es][:, m, off : off + ln],
                            start=(es == 0 and m == 0),
                            stop=(es == 1 and m == MH - 1),
                        )
        yt_sb = sb.tile([128, KD, SUPER], MM_DT, tag="ytsb")
        nc.vector.tensor_copy(yt_sb[:], yt_ps[:])

        # --- output projection + dense permuted store
        o_sb = sb.tile([128, SUPER // 128, EMB], MM_DT, tag="osb")
        for g4 in range(SUPER // 128):
            o_ps = ps.tile([128, EMB], F32, tag="o", bufs=1)
            for kd in range(KD):
                nc.tensor.matmul(
                    o_ps[:],
                    yt_sb[:, kd, g4 * 128 : (g4 + 1) * 128],
                    wp_sb[:, kd, :],
                    start=(kd == 0),
                    stop=(kd == KD - 1),
                )
            if g4 == 0:
                nc.scalar.copy(o_sb[:, g4, :], o_ps[:])
            else:
                nc.vector.tensor_copy(o_sb[:, g4, :], o_ps[:])
        nc.sync.dma_start(
            pout[sl0 : sl0 + SUPER, :].rearrange("(g p) e -> p g e", p=128), o_sb[:]
        )


_PROGRAM = None


def _build(num_devices=NCORES):
    global _PROGRAM
    if _PROGRAM is not None:
        return _PROGRAM
    nc = bacc.Bacc(
        "TRN2", target_bir_lowering=False, debug=False, num_devices=num_devices
    )
    xt = nc.dram_tensor("xt", [D, NSH], F32, kind="ExternalInput").ap()
    xbf = nc.dram_tensor("xbf", [NSH, D], MM_DT, kind="ExternalInput").ap()
    wg = nc.dram_tensor("w_gate", [D, E], F32, kind="ExternalInput").ap()
    w1 = nc.dram_tensor("w1", [E, D, H], MM_DT, kind="ExternalInput").ap()
    w2 = nc.dram_tensor("w2", [E, H, D], MM_DT, kind="ExternalInput").ap()
    wp = nc.dram_tensor("w_proj", [D, EMB], MM_DT, kind="ExternalInput").ap()
    u128 = nc.dram_tensor("u128", [128, 128], F32, kind="ExternalInput").ap()
    ident = nc.dram_tensor("ident", [128, 128], F32, kind="ExternalInput").ap()
    r16 = nc.dram_tensor("r16", [16, 128], F32, kind="ExternalInput").ap()
    pmeta = nc.dram_tensor("perm_meta", [CAP_TOT, 64], F32, kind="ExternalOutput").ap()
    pout = nc.dram_tensor("perm_out", [CAP_TOT, EMB], MM_DT, kind="ExternalOutput").ap()
    with tile.TileContext(nc) as tc, ExitStack() as ctx:
        _moe_body(ctx, tc, xt, xbf, wg, w1, w2, wp, u128, ident, r16, pmeta, pout)
    nc.compile()
    _PROGRAM = nc
    return nc


def _make_in_map(x, w_gate, w1, w2, w_proj, i):
    import ml_dtypes

    bf16 = ml_dtypes.bfloat16
    xs = x[i * NSH : (i + 1) * NSH]
    return {
        "xt": np.ascontiguousarray(xs.T),
        "xbf": np.ascontiguousarray(xs.astype(bf16)),
        "w_gate": np.ascontiguousarray(w_gate),
        "w1": np.ascontiguousarray(w1.astype(bf16)),
        "w2": np.ascontiguousarray(w2.astype(bf16)),
        "w_proj": np.ascontiguousarray(w_proj.astype(bf16)),
        "u128": np.triu(np.ones((128, 128), np.float32), 1),
        "ident": np.eye(128, dtype=np.float32),
        "r16": (np.arange(128)[None, :] % 16 == np.arange(16)[:, None]).astype(
            np.float32
        ),
    }


def _unpermute(pm, po, out_chunk):
    tok = np.rint(pm[:, 0]).astype(np.int64) - 1
    valid = tok >= 0
    tv = tok[valid]
    assert tv.size == NSH, f"expected {NSH} routed tokens, got {tv.size}"
    assert np.unique(tv).size == NSH, "duplicate token slots after routing"
    out_chunk[tv] = po[valid].astype(np.float32)


def _install_trace_shim():
    """Recreate the antenv.axon_hooks NTFF profile hook (missing in this image)."""
    import sys
    import types
    import contextlib
    import ctypes

    if "antenv.axon_hooks" in sys.modules:
        return
    so_path = "/opt/axon/libaxon_pjrt.so"
    lib = ctypes.CDLL(so_path)
    lib.axon_start_nrt_profile.argtypes = [ctypes.POINTER(ctypes.c_int64), ctypes.c_size_t]
    lib.axon_start_nrt_profile.restype = ctypes.c_int64
    lib.axon_stop_nrt_profile.argtypes = [ctypes.c_char_p]
    lib.axon_stop_nrt_profile.restype = ctypes.c_int64

    @contextlib.contextmanager
    def _hook(output_dir, device_ids):
        import jax

        jax.devices()
        if device_ids:
            ids = (ctypes.c_int64 * len(device_ids))(*device_ids)
            rc = lib.axon_start_nrt_profile(ids, len(device_ids))
        else:
            rc = lib.axon_start_nrt_profile(None, 0)
        if rc != 0:
            raise RuntimeError(f"axon_start_nrt_profile rc={rc}")
        try:
            yield
        finally:
            n = lib.axon_stop_nrt_profile(str(output_dir).encode())
            if n <= 0:
                print(f"profile: {n} ntff files written to {output_dir}")

    mod = types.ModuleType("antenv.axon_hooks")
    _state = {"hook": _hook}
    mod.get_axon_ntff_profile_hook = lambda: _state["hook"]
    mod.set_axon_ntff_profile_hook = lambda h: _state.__setitem__("hook", h)
    sys.modules["antenv.axon_hooks"] = mod

    import concourse.bass_utils as bu

    bu.upload_artifacts = lambda tmpdir: f"local:{tmpdir}"


def kernel(x, w_gate, w1, w2, w_proj, b_proj):
    nc = _build()
    in_maps = [_make_in_map(x, w_gate, w1, w2, w_proj, i) for i in range(NCORES)]
    trace = bool(int(os.environ.get("MOE_TRACE", "0")))
    if trace:
        _install_trace_shim()
        import tempfile

        tmpdir = os.environ.get("MOE_TRACE_DIR") or tempfile.mkdtemp(prefix="moe_trace_")
        res = run_bass_kernel_spmd(
            nc, in_maps, list(range(NCORES)), trace=True, tmpdir=tmpdir,
            trace_cores=[0],
        )
        print(f"HW exec time: {res.exec_time_ns} ns")
        print(f"trace dir: {tmpdir}")
        kernel.last_result = res
    else:
        res = run_bass_kernel_spmd(nc, in_maps, list(range(NCORES)))
    out = np.empty((N, EMB), np.float32)
    for i in range(NCORES):
        _unpermute(
            res.results[i]["perm_meta"],
            res.results[i]["perm_out"],
            out[i * NSH : (i + 1) * NSH],
        )
    return out + b_proj[None, :]


# revision 4
# speedup vs baseline: 1.6760x; 1.6760x over previous
"""Trainium2 Bass kernel for MoE head adapter (top-2 of 4 experts + proj).

Sparse expert-pair dispatch version.

Per core (8192 tokens, weights replicated, data-parallel over 8 cores):
  Phase A (gating+routing, batched over all 64 token-blocks):
    - exact-f32 gating logits via PE matmuls (tokens on partitions)
    - top-2 mask, softmax-of-2 -> hi-expert gate (lo = 1 - hi)
    - pair key in {1..6} via dot(mask, [0,1,2,4])
    - global rank within pair region: inclusive scan along free axis
      (tensor_tensor_scan) + strict-upper-triangular matmul for the
      partition prefix; slot = region_base - 1 + prefix + incl_scan
    - one indirect DMA scatters (token_id, hi_gate) f32 pairs into
      perm_meta[slot] (pads stay -1 from an init fill)
  Phase B (19 supertiles of 512 slots, pair regions 256-aligned):
    - load meta block; idx16 = max(token,0) wrap-16 replicated
    - dma_gather(transpose=True): gathers 512 token rows of x (bf16)
      AND transposes into [128, kd, 512] xT layout in one DMA
    - G_hi broadcast via ones-matmul; G_lo = 1 - G_hi
    - 2-expert up-proj / gelu / gate-mult / down-proj (bf16 matmuls),
      column-split at the <=1 pair boundary inside a supertile
    - output projection; dense bf16 write to perm_out[slot]
  Host: unpermute perm_out rows via perm_meta token ids, upcast, + b_proj.
"""

import os
from contextlib import ExitStack

import numpy as np

import concourse.bass as bass
import concourse.tile as tile
from concourse import bacc, mybir
from concourse.bass import IndirectOffsetOnAxis
from concourse.bass_utils import run_bass_kernel_spmd

N, D, E, H, EMB = 65536, 256, 4, 512, 512
NCORES = 8
NSH = N // NCORES           # tokens per core
KD = D // 128               # k-tiles over D
MH = H // 128               # m-tiles over H
SUPER = 512
NSUP = NSH // SUPER         # gating supertiles (16)
NSB = NSH // 128            # 128-token blocks (64)

# expert-pair regions, ordered by pair key (dot(mask, [0,1,2,4]) = j+1)
PAIRS = [(0, 1), (0, 2), (1, 2), (0, 3), (1, 3), (2, 3)]
CAPS = [2304, 768, 1792, 1792, 768, 2304]
STARTS = [0]
for c in CAPS:
    STARTS.append(STARTS[-1] + c)
CAP_TOT = STARTS[-1]        # 9728
NST = CAP_TOT // SUPER      # 19 supertiles


def _runs_for_st(s):
    """[(pair_j, col_off, col_len)] covering slots [512s, 512s+512)."""
    lo, hi = s * SUPER, (s + 1) * SUPER
    runs = []
    for j in range(6):
        a, b = max(lo, STARTS[j]), min(hi, STARTS[j + 1])
        if a < b:
            runs.append((j, a - lo, b - a))
    return runs


RUNS = [_runs_for_st(s) for s in range(NST)]

F32 = mybir.dt.float32
I32 = mybir.dt.int32
I16 = mybir.dt.int16
BF16 = mybir.dt.bfloat16
AF = mybir.ActivationFunctionType
ALU = mybir.AluOpType
AX = mybir.AxisListType
MM_DT = BF16


def _moe_body(ctx: ExitStack, tc, xt, xbf, wg, w1, w2, wp, u128, ident, r16, px, pout):
    nc = tc.nc

    const = ctx.enter_context(tc.tile_pool(name="const", bufs=1))
    sb = ctx.enter_context(tc.tile_pool(name="sb", bufs=2))
    ps = ctx.enter_context(tc.tile_pool(name="ps", bufs=1, space="PSUM"))

    # --- replicated weights (HWDGE; keep the gpsimd Q7 free for the scatter)
    w1_sb = const.tile([128, KD, E, H], MM_DT)
    w2_sb = const.tile([128, MH, E, D], MM_DT)
    wp_sb = const.tile([128, KD, EMB], MM_DT)
    wg_sb = const.tile([128, KD, E], F32)
    for k in range(KD):
        for e in range(E):
            nc.sync.dma_start(w1_sb[:, k, e, :], w1[e, k * 128:(k + 1) * 128, :])
    for m in range(MH):
        for e in range(E):
            nc.sync.dma_start(w2_sb[:, m, e, :], w2[e, m * 128:(m + 1) * 128, :])
    nc.sync.dma_start(wp_sb[:], wp.rearrange("(k p) m -> p k m", p=128))
    nc.sync.dma_start(wg_sb[:], wg.rearrange("(k p) e -> p k e", p=128))
    u128_sb = const.tile([128, 128], F32)
    nc.sync.dma_start(u128_sb[:], u128[:])
    ident_sb = const.tile([128, 128], F32)
    nc.sync.dma_start(ident_sb[:], ident[:])
    r16_sb = const.tile([16, 128], F32)
    nc.sync.dma_start(r16_sb[:], r16[:])

    # --- small constants ----------------------------------------------------
    ones_bf = const.tile([1, 128], MM_DT)
    nc.vector.memset(ones_bf[:], 1.0)
    ones_f = const.tile([1, 128], F32)
    nc.vector.memset(ones_f[:], 1.0)
    basev = const.tile([1, 6], F32)
    for j in range(6):
        nc.vector.memset(basev[:, j : j + 1], float(STARTS[j] - 1))
    cvals = const.tile([128, 1, 4], F32)
    for e, cv in enumerate((0.0, 1.0, 2.0, 4.0)):
        nc.vector.memset(cvals[:, :, e : e + 1], cv)
    # token (s+1, p) coordinates, bf16-exact; zero rows decode as pad
    s1_i = const.tile([128, NSB], I32)
    nc.gpsimd.iota(s1_i[:], pattern=[[1, NSB]], base=1, channel_multiplier=0)
    s1_bf = const.tile([128, NSB], MM_DT)
    nc.vector.tensor_copy(s1_bf[:], s1_i[:])
    p_i = const.tile([128, NSB], I32)
    nc.gpsimd.iota(p_i[:], pattern=[[0, NSB]], base=0, channel_multiplier=1)
    p_bf = const.tile([128, NSB], MM_DT)
    nc.vector.tensor_copy(p_bf[:], p_i[:])
    ev_i = const.tile([128, 4], I32)
    nc.gpsimd.iota(ev_i[:], pattern=[[1, 4]], base=0, channel_multiplier=0)
    evals = const.tile([128, 1, 4], F32)
    nc.vector.tensor_copy(evals[:, 0, :], ev_i[:])
    jv_i = const.tile([128, 6], I32)
    nc.gpsimd.iota(jv_i[:], pattern=[[1, 6]], base=1, channel_multiplier=0)
    jvals = const.tile([128, 6, 1], F32)
    nc.vector.tensor_copy(jvals[:, :, 0], jv_i[:])

    # --- zero-init perm_x (scatter-add target; pad rows stay 0) -------------
    PXW = 384  # perm_x row width (768B stride, 259 used)
    zw = CAP_TOT * PXW // 128 // 8
    zin = const.tile([128, zw], MM_DT)
    nc.vector.memset(zin[:], 0.0)
    px_flat = px.rearrange("(p a) c -> p (a c)", p=128)
    for b in range(8):
        nc.sync.dma_start(px_flat[:, b * zw : (b + 1) * zw], zin[:])

    # --- permuted x rows: [x(256) | hi_gate | s+1 | p] bf16 -----------------
    xrow = const.tile([128, NSB, 259], MM_DT)
    nc.sync.dma_start(xrow[:, :, 0:256], xbf.rearrange("(s p) d -> p s d", p=128))
    nc.vector.tensor_copy(xrow[:, :, 257], s1_bf[:])
    nc.vector.tensor_copy(xrow[:, :, 258], p_bf[:])

    # ===================== Phase A: gating + routing =========================
    lg_ps = ps.tile([128, NSB, E], F32, tag="h", bufs=2)
    for T in range(NSUP):
        xt32 = sb.tile([128, KD, SUPER], F32, tag="xt32", bufs=3)
        nc.sync.dma_start(
            xt32[:],
            xt[:, T * SUPER : (T + 1) * SUPER].rearrange("(k p) t -> p k t", p=128),
        )
        for s4 in range(SUPER // 128):
            s = T * 4 + s4
            for k in range(KD):
                nc.tensor.matmul(
                    lg_ps[:, s, :],
                    xt32[:, k, s4 * 128 : (s4 + 1) * 128],
                    wg_sb[:, k, :],
                    start=(k == 0),
                    stop=(k == KD - 1),
                )

    def bc(t, shape):
        return t[:].broadcast_to(shape)

    SH = [128, NSB, E]
    lg = sb.tile(SH, F32, tag="lg_sb")
    nc.vector.tensor_copy(lg[:], lg_ps[:])
    m1 = sb.tile([128, NSB, 1], F32, tag="m1")
    nc.vector.reduce_max(m1[:], lg[:], axis=AX.X)
    t0 = sb.tile(SH, F32, tag="t0")
    nc.vector.tensor_tensor(t0[:], lg[:], bc(m1, SH), op=ALU.is_equal)
    t1 = sb.tile(SH, F32, tag="t1")
    nc.vector.tensor_scalar_mul(t1[:], t0[:], -1e9)
    t2 = sb.tile(SH, F32, tag="t2")
    nc.vector.tensor_tensor(t2[:], lg[:], t1[:], op=ALU.add)
    m2 = sb.tile([128, NSB, 1], F32, tag="m2")
    nc.vector.reduce_max(m2[:], t2[:], axis=AX.X)
    mk = sb.tile(SH, F32, tag="mk")
    nc.vector.tensor_tensor(mk[:], lg[:], bc(m2, SH), op=ALU.is_ge)
    t4 = sb.tile(SH, F32, tag="t4")
    nc.vector.tensor_tensor(t4[:], lg[:], bc(m1, SH), op=ALU.subtract)
    t5 = sb.tile(SH, F32, tag="t5")
    nc.scalar.activation(t5[:], t4[:], AF.Exp)
    t6 = sb.tile(SH, F32, tag="t6")
    nc.vector.tensor_tensor(t6[:], t5[:], mk[:], op=ALU.mult)
    den = sb.tile([128, NSB, 1], F32, tag="den")
    nc.vector.reduce_sum(den[:], t6[:], axis=AX.X)
    rcp = sb.tile([128, NSB, 1], F32, tag="rcp")
    nc.vector.reciprocal(rcp[:], den[:])
    # hi-expert (higher index of the selected pair) gate
    em = sb.tile(SH, F32, tag="em")
    nc.vector.tensor_tensor(em[:], mk[:], bc(evals, SH), op=ALU.mult)
    bh = sb.tile([128, NSB, 1], F32, tag="bh")
    nc.vector.reduce_max(bh[:], em[:], axis=AX.X)
    hm = sb.tile(SH, F32, tag="hm")
    nc.vector.tensor_tensor(hm[:], em[:], bc(bh, SH), op=ALU.is_equal)
    t7 = sb.tile(SH, F32, tag="t7")
    nc.vector.tensor_tensor(t7[:], t6[:], hm[:], op=ALU.mult)
    hs = sb.tile([128, NSB, 1], F32, tag="hs")
    nc.vector.reduce_sum(hs[:], t7[:], axis=AX.X)
    hig = sb.tile([128, NSB, 1], F32, tag="hig")
    nc.vector.tensor_tensor(hig[:], hs[:], rcp[:], op=ALU.mult)
    # pair key (1..6)
    t8 = sb.tile(SH, F32, tag="t8")
    nc.vector.tensor_tensor(t8[:], mk[:], bc(cvals, SH), op=ALU.mult)
    pk = sb.tile([128, NSB, 1], F32, tag="pk")
    nc.vector.reduce_sum(pk[:], t8[:], axis=AX.X)
    # pair indicator [128, 6, NSB]
    IS = [128, 6, NSB]
    I_t = sb.tile(IS, F32, tag="I")
    nc.vector.tensor_tensor(
        I_t[:],
        pk[:, None, :, 0].broadcast_to(IS),
        bc(jvals, IS),
        op=ALU.is_equal,
    )
    # inclusive scan along the free (block) axis, per pair lane
    incl = sb.tile(IS, F32, tag="incl")
    for j in range(6):
        nc.vector.tensor_tensor_scan(
            incl[:, j, :], I_t[:, j, :], I_t[:, j, :], 0.0, op0=ALU.add, op1=ALU.bypass
        )
    # partition-exclusive prefix of lane rowsums + (base - 1)
    pfx_ps = ps.tile([128, 6], F32, tag="G")
    nc.tensor.matmul(pfx_ps[:], u128_sb[:], incl[:, :, NSB - 1], start=True, stop=False)
    nc.tensor.matmul(pfx_ps[:], ones_f[:], basev[:], start=False, stop=True)
    pfx = sb.tile([128, 6], F32, tag="pfxsb")
    nc.vector.tensor_copy(pfx[:], pfx_ps[:])
    sstuff = sb.tile(IS, F32, tag="sstuff")
    nc.vector.tensor_tensor(
        sstuff[:], incl[:], pfx[:, :, None].broadcast_to(IS), op=ALU.add
    )
    sel = sb.tile(IS, F32, tag="sel")
    nc.vector.tensor_tensor(sel[:], sstuff[:], I_t[:], op=ALU.mult)
    s3 = sb.tile([128, 3, NSB], F32, tag="s3")
    nc.vector.tensor_tensor(s3[:], sel[:, 0:3, :], sel[:, 3:6, :], op=ALU.add)
    s2 = sb.tile([128, NSB], F32, tag="s2")
    nc.vector.tensor_tensor(s2[:], s3[:, 0, :], s3[:, 1, :], op=ALU.add)
    slot_f = sb.tile([128, NSB], F32, tag="slotf")
    nc.vector.tensor_tensor(slot_f[:], s2[:], s3[:, 2, :], op=ALU.add)
    nc.vector.tensor_copy(xrow[:, :, 256], hig[:, :, 0])
    # Build the scatter index list in 16-partition wrap order: position
    # i = s*128 + p must sit at [i%16, i//16] = [p%16, s*8 + p//16].
    # Two transpose stages move slot[p, s] -> idxT[q, j, s] (p = j*16+q),
    # then a strided copy reorders (j, s) -> (s, j), and an R16 matmul
    # replicates the 16 live partitions across all 128.
    slotT_ps = ps.tile([64, 128], F32, tag="G")
    nc.tensor.transpose(slotT_ps[:], slot_f[:], ident_sb[:])
    slotT = sb.tile([64, 128], F32, tag="slotT")
    nc.vector.tensor_copy(slotT[:], slotT_ps[:])
    idxT_ps = ps.tile([16, 8, NSB], F32, tag="o")
    for j in range(8):
        nc.tensor.transpose(
            idxT_ps[:, j, :], slotT[:, j * 16 : (j + 1) * 16], ident_sb[:64, :64]
        )
    idx_sw = sb.tile([16, NSB, 8], F32, tag="idxsw")
    nc.vector.tensor_copy(idx_sw[:], idxT_ps[:].rearrange("q j s -> q s j"))
    rep_ps = ps.tile([128, NSB * 8], F32, tag="h", bufs=2)
    nc.tensor.matmul(
        rep_ps[:], r16_sb[:], idx_sw[:].rearrange("q s j -> q (s j)"),
        start=True, stop=True,
    )
    idx16s = sb.tile([128, NSB * 8], I16, tag="idx16s")
    nc.vector.tensor_copy(idx16s[:], rep_ps[:])
    # SWDGE ring fits <=128 descriptors per direction per op (m2s = n/8+1),
    # so scatter in chunks of 896 rows (7 s-columns each).
    CH = 7
    for c0 in range(0, NSB, CH):
        cw = min(CH, NSB - c0)
        n = cw * 128
        nc.gpsimd.dma_scatter_add(
            px[:, 0:259],
            xrow[:, c0 : c0 + cw, :],
            idx16s[:, c0 * 8 : (c0 + cw) * 8],
            n,
            n,
            259,
            elem_step=PXW,
        )

    # ===================== Phase B: experts + proj ===========================
    for s in range(NST):
        sl0 = s * SUPER
        # --- permuted xT via HWDGE xbar-transpose loads (no gpsimd involved)
        xt_sb = sb.tile([128, KD, SUPER], MM_DT, tag="xtg", bufs=3)
        for k in range(KD):
            nc.sync.dma_start_transpose(
                xt_sb[:, k, :], px[sl0 : sl0 + SUPER, k * 128 : (k + 1) * 128]
            )
        hi_bf = sb.tile([1, SUPER], MM_DT, tag="hibf")
        nc.sync.dma_start(
            hi_bf[:], px[sl0 : sl0 + SUPER, 256:257].rearrange("t one -> one t")
        )
        # --- token gates broadcast over partitions
        g_ps = ps.tile([128, SUPER], F32, tag="G")
        nc.tensor.matmul(g_ps[:], ones_bf[:], hi_bf[:], start=True, stop=True)
        g_hi = sb.tile([128, SUPER], MM_DT, tag="ghi")
        nc.vector.tensor_copy(g_hi[:], g_ps[:])
        g_lo = sb.tile([128, SUPER], MM_DT, tag="glo")
        nc.vector.tensor_scalar(g_lo[:], g_ps[:], -1.0, 1.0, op0=ALU.mult, op1=ALU.add)
        g_es = (g_lo, g_hi)

        # --- experts (2 per token, split at pair boundaries)
        hgg = [
            sb.tile([128, MH, SUPER], MM_DT, tag=f"hgg{es}", name=f"hgg{s}_{es}")
            for es in range(2)
        ]
        for j, off, ln in RUNS[s]:
            pair = PAIRS[j]
            for es in range(2):
                e = pair[es]
                for mp in range(MH // 2):
                    h_ps = ps.tile(
                        [128, 2, SUPER], F32, tag="h", bufs=2, name=f"h{s}_{j}_{es}_{mp}"
                    )
                    for mm in range(2):
                        m = 2 * mp + mm
                        for k in range(KD):
                            nc.tensor.matmul(
                                h_ps[:, mm, off : off + ln],
                                w1_sb[:, k, e, m * 128 : (m + 1) * 128],
                                xt_sb[:, k, off : off + ln],
                                start=(k == 0),
                                stop=(k == KD - 1),
                            )
                    hg = sb.tile([128, 2, SUPER], MM_DT, tag="hg", bufs=3)
                    nc.scalar.activation(
                        hg[:, :, off : off + ln], h_ps[:, :, off : off + ln], AF.Gelu
                    )
                    nc.vector.tensor_tensor(
                        hgg[es][:, 2 * mp : 2 * mp + 2, off : off + ln],
                        hg[:, :, off : off + ln],
                        g_es[es][:, None, off : off + ln].broadcast_to([128, 2, ln]),
                        op=ALU.mult,
                    )
        yt_ps = ps.tile([128, KD, SUPER], F32, tag="yt")
        nruns = len(RUNS[s])
        for ri, (j, off, ln) in enumerate(RUNS[s]):
            pair = PAIRS[j]
            for es in range(2):
                e = pair[es]
                for md in range(KD):
                    for m in range(MH):
                        nc.tensor.matmul(
                            yt_ps[:, md, off : off + ln],
                            w2_sb[:, m, e, md * 128 : (md + 1) * 128],
                            hgg[es][:, m, off : off + ln],
                            start=(es == 0 and m == 0),
                            stop=(es == 1 and m == MH - 1),
                        )
        yt_sb = sb.tile([128, KD, SUPER], MM_DT, tag="ytsb")
        nc.vector.tensor_copy(yt_sb[:], yt_ps[:])

        # --- output projection + dense permuted store
        o_sb = sb.tile([128, SUPER // 128, EMB], MM_DT, tag="osb")
        for g4 in range(SUPER // 128):
            o_ps = ps.tile([128, EMB], F32, tag="o", bufs=1)
            for kd in range(KD):
                nc.tensor.matmul(
                    o_ps[:],
                    yt_sb[:, kd, g4 * 128 : (g4 + 1) * 128],
                    wp_sb[:, kd, :],
                    start=(kd == 0),
                    stop=(kd == KD - 1),
                )
            if g4 == 0:
                nc.scalar.copy(o_sb[:, g4, :], o_ps[:])
            else:
                nc.vector.tensor_copy(o_sb[:, g4, :], o_ps[:])
        nc.sync.dma_start(
            pout[sl0 : sl0 + SUPER, :].rearrange("(g p) e -> p g e", p=128), o_sb[:]
        )


_PROGRAM = None


def _build(num_devices=NCORES):
    global _PROGRAM
    if _PROGRAM is not None:
        return _PROGRAM
    nc = bacc.Bacc(
        "TRN2", target_bir_lowering=False, debug=False, num_devices=num_devices
    )
    xt = nc.dram_tensor("xt", [D, NSH], F32, kind="ExternalInput").ap()
    xbf = nc.dram_tensor("xbf", [NSH, D], MM_DT, kind="ExternalInput").ap()
    wg = nc.dram_tensor("w_gate", [D, E], F32, kind="ExternalInput").ap()
    w1 = nc.dram_tensor("w1", [E, D, H], MM_DT, kind="ExternalInput").ap()
    w2 = nc.dram_tensor("w2", [E, H, D], MM_DT, kind="ExternalInput").ap()
    wp = nc.dram_tensor("w_proj", [D, EMB], MM_DT, kind="ExternalInput").ap()
    u128 = nc.dram_tensor("u128", [128, 128], F32, kind="ExternalInput").ap()
    ident = nc.dram_tensor("ident", [128, 128], F32, kind="ExternalInput").ap()
    r16 = nc.dram_tensor("r16", [16, 128], F32, kind="ExternalInput").ap()
    px = nc.dram_tensor("perm_x", [CAP_TOT, 384], MM_DT, kind="ExternalOutput").ap()
    pout = nc.dram_tensor("perm_out", [CAP_TOT, EMB], MM_DT, kind="ExternalOutput").ap()
    with tile.TileContext(nc) as tc, ExitStack() as ctx:
        _moe_body(ctx, tc, xt, xbf, wg, w1, w2, wp, u128, ident, r16, px, pout)
    nc.compile()
    _PROGRAM = nc
    return nc


def _make_in_map(x, w_gate, w1, w2, w_proj, i):
    import ml_dtypes

    bf16 = ml_dtypes.bfloat16
    xs = x[i * NSH : (i + 1) * NSH]
    return {
        "xt": np.ascontiguousarray(xs.T),
        "xbf": np.ascontiguousarray(xs.astype(bf16)),
        "w_gate": np.ascontiguousarray(w_gate),
        "w1": np.ascontiguousarray(w1.astype(bf16)),
        "w2": np.ascontiguousarray(w2.astype(bf16)),
        "w_proj": np.ascontiguousarray(w_proj.astype(bf16)),
        "u128": np.triu(np.ones((128, 128), np.float32), 1),
        "ident": np.eye(128, dtype=np.float32),
        "r16": (np.arange(128)[None, :] % 16 == np.arange(16)[:, None]).astype(
            np.float32
        ),
    }


def _unpermute(px_arr, po, out_chunk):
    s1 = np.rint(px_arr[:, 257].astype(np.float32)).astype(np.int64)
    p = np.rint(px_arr[:, 258].astype(np.float32)).astype(np.int64)
    valid = s1 >= 1
    tok = (s1[valid] - 1) * 128 + p[valid]
    assert tok.size == NSH, f"expected {NSH} routed tokens, got {tok.size}"
    assert np.unique(tok).size == NSH, "duplicate token slots after routing"
    out_chunk[tok] = po[valid].astype(np.float32)


def _install_trace_shim():
    """Recreate the antenv.axon_hooks NTFF profile hook (missing in this image)."""
    import sys
    import types
    import contextlib
    import ctypes

    if "antenv.axon_hooks" in sys.modules:
        return
    so_path = "/opt/axon/libaxon_pjrt.so"
    lib = ctypes.CDLL(so_path)
    lib.axon_start_nrt_profile.argtypes = [ctypes.POINTER(ctypes.c_int64), ctypes.c_size_t]
    lib.axon_start_nrt_profile.restype = ctypes.c_int64
    lib.axon_stop_nrt_profile.argtypes = [ctypes.c_char_p]
    lib.axon_stop_nrt_profile.restype = ctypes.c_int64

    @contextlib.contextmanager
    def _hook(output_dir, device_ids):
        import jax

        jax.devices()
        if device_ids:
            ids = (ctypes.c_int64 * len(device_ids))(*device_ids)
            rc = lib.axon_start_nrt_profile(ids, len(device_ids))
        else:
            rc = lib.axon_start_nrt_profile(None, 0)
        if rc != 0:
            raise RuntimeError(f"axon_start_nrt_profile rc={rc}")
        try:
            yield
        finally:
            n = lib.axon_stop_nrt_profile(str(output_dir).encode())
            if n <= 0:
                print(f"profile: {n} ntff files written to {output_dir}")

    mod = types.ModuleType("antenv.axon_hooks")
    _state = {"hook": _hook}
    mod.get_axon_ntff_profile_hook = lambda: _state["hook"]
    mod.set_axon_ntff_profile_hook = lambda h: _state.__setitem__("hook", h)
    sys.modules["antenv.axon_hooks"] = mod

    import concourse.bass_utils as bu

    bu.upload_artifacts = lambda tmpdir: f"local:{tmpdir}"


def kernel(x, w_gate, w1, w2, w_proj, b_proj):
    nc = _build()
    in_maps = [_make_in_map(x, w_gate, w1, w2, w_proj, i) for i in range(NCORES)]
    trace = bool(int(os.environ.get("MOE_TRACE", "0")))
    if trace:
        _install_trace_shim()
        import tempfile

        tmpdir = os.environ.get("MOE_TRACE_DIR") or tempfile.mkdtemp(prefix="moe_trace_")
        res = run_bass_kernel_spmd(
            nc, in_maps, list(range(NCORES)), trace=True, tmpdir=tmpdir,
            trace_cores=[0],
        )
        print(f"HW exec time: {res.exec_time_ns} ns")
        print(f"trace dir: {tmpdir}")
        kernel.last_result = res
    else:
        res = run_bass_kernel_spmd(nc, in_maps, list(range(NCORES)))
    out = np.empty((N, EMB), np.float32)
    for i in range(NCORES):
        _unpermute(
            res.results[i]["perm_x"],
            res.results[i]["perm_out"],
            out[i * NSH : (i + 1) * NSH],
        )
    return out + b_proj[None, :]
